# revision 17
# baseline (speedup 1.0000x reference)
"""Trainium2 Bass kernel for ChannelCrissCrossAttention.

Shapes (hardcoded): B=8, IN=128, C=16, V=T=64.
Sharding: pure data parallel, one batch element per NeuronCore (8 cores).

Math notes (derived from the reference's reshape/view semantics):
  q,k,v = conv3x3(x; wq/wk/wv)  -> [C, V, T] per batch element.
  - tt branch:  e_tt[c,v,t,j] = q[c,v,t] * k[c,v,j]
  - vv branch:  e_vv[c,v,t,j] = q[c,t,v] * k[c,t,j]  == e_tt[c,t,v,j]
    (the reference's .reshape on a [B,C,V,T] buffer makes vv the exact
     spatial transpose of tt, and its value rows coincide as well), so
    one pass over rows r=(c,y) gives both branches:
      G_r[x,j] = exp(qr[x]*kr[j]);  S_r[x] = sum_j G;  W_r[x] = sum_j G*vr[j]
  - cc branch: with rows r'=(t,v) over the flat [C*V*T] buffers
    (qf chunked by 16): logits qf[r'*16+c]*qf-like kf[r'*16+j], value
    chunk vf[(v*64+t)*16+j].  Computed here in a [t-partition, (v,c)]
    layout so the per-block (c,v,t) outputs fall out of a single PE
    transpose per block (no 4-byte-element DMA transposes).
  Z = S_tt + S_vv + S_cc (joint softmax denominator); the three outputs are
  W_*/Z, then one stacked reverse conv (48->128) + gamma*out + x.

Convs run in bf16 (PSUM accumulates in f32); everything else f32.
"""

import sys

sys.path.insert(0, "/opt/trn_rl_repo")

import ml_dtypes
import numpy as np

import concourse.bass as bass
import concourse.tile as tile
from concourse import bacc, mybir
from concourse.bass_utils import run_bass_kernel_spmd

F32 = mybir.dt.float32
BF16 = mybir.dt.bfloat16
AF = mybir.ActivationFunctionType
ALU = mybir.AluOpType
AX = mybir.AxisListType

IN, C, V, T = 128, 16, 64, 64
CH3 = 3 * C  # 48
NPOS = V * T  # 4096
PW = 66  # padded spatial width
NPBF = np.dtype(ml_dtypes.bfloat16)


def _build_program(niter=1):
    nc = bacc.Bacc("TRN2", target_bir_lowering=False, debug=False)

    x_d = nc.dram_tensor("x", [IN, V, T], F32, kind="ExternalInput")
    xbf_d = nc.dram_tensor("xbf", [IN, V, T], BF16, kind="ExternalInput")
    wqkv_d = nc.dram_tensor("wqkv", [IN, 9 * CH3], BF16, kind="ExternalInput")
    bqkv_d = nc.dram_tensor("bqkv", [CH3, 1], F32, kind="ExternalInput")
    wr_d = nc.dram_tensor("wr", [CH3, 9 * IN], BF16, kind="ExternalInput")
    gb_d = nc.dram_tensor("gb", [IN, 1], F32, kind="ExternalInput")
    id_d = nc.dram_tensor("ident", [2 * T, T], F32, kind="ExternalInput")
    out_d = nc.dram_tensor("out", [IN, V, T], F32, kind="ExternalOutput")

    # HBM bounce buffers (SBUF->SBUF DMAs cannot cross the partition/free
    # axis boundary, so layout changes bounce through DRAM)
    qkv_h = nc.dram_tensor("qkv_h", [CH3, V, T], F32)
    ocat_h = nc.dram_tensor("ocat_h", [CH3, V, T], F32)

    with tile.TileContext(nc) as tc:
        if niter == 1:
            _body(nc, tc, x_d, xbf_d, wqkv_d, bqkv_d, wr_d, gb_d, id_d, out_d,
                  qkv_h, ocat_h)
        else:
            # hardware loop: the N-iteration program has the same
            # instruction count as 1-iteration, so the (tN - t1) wall
            # delta isolates true marginal device-execution time instead
            # of NEFF load/dispatch overhead (which scales with the
            # instruction stream size).
            with tc.For_i(0, niter):
                _body(nc, tc, x_d, xbf_d, wqkv_d, bqkv_d, wr_d, gb_d, id_d,
                      out_d, qkv_h, ocat_h)

    nc.compile()
    return nc


def _body(nc, tc, x_d, xbf_d, wqkv_d, bqkv_d, wr_d, gb_d, id_d, out_d,
          qkv_h, ocat_h):
    from contextlib import ExitStack
    ctx = ExitStack()
    persist = ctx.enter_context(tc.tile_pool(name="persist", bufs=1))
    pio = ctx.enter_context(tc.tile_pool(name="pio", bufs=2))
    pqkv = ctx.enter_context(tc.tile_pool(name="pqkv", bufs=2))
    pP = ctx.enter_context(tc.tile_pool(name="pP", bufs=2))
    pG = ctx.enter_context(tc.tile_pool(name="pG", bufs=2))
    pcomb = ctx.enter_context(tc.tile_pool(name="pcomb", bufs=2))
    psum1 = ctx.enter_context(
        tc.tile_pool(name="psum1", bufs=2, space=bass.MemorySpace.PSUM))
    psum2 = ctx.enter_context(
        tc.tile_pool(name="psum2", bufs=2, space=bass.MemorySpace.PSUM))
    psumT = ctx.enter_context(
        tc.tile_pool(name="psumT", bufs=1, space=bass.MemorySpace.PSUM))

    # ---- Phase 0: weights + padded bf16 input + f32 residual copy ----
    wqkv = persist.tile([IN, 9 * CH3], BF16)
    nc.sync.dma_start(wqkv[:], wqkv_d.ap())
    bqkv = persist.tile([CH3, 1], F32)
    nc.sync.dma_start(bqkv[:], bqkv_d.ap())
    wr = persist.tile([CH3, 9 * IN], BF16)
    nc.sync.dma_start(wr[:], wr_d.ap())
    gb = persist.tile([IN, 1], F32)
    nc.sync.dma_start(gb[:], gb_d.ap())
    ident = persist.tile([2 * T, T], F32)
    nc.sync.dma_start(ident[:], id_d.ap())

    xpad = persist.tile([IN, PW * PW], BF16)
    nc.gpsimd.memset(xpad[:], 0.0)
    xpad_v = xpad[:].rearrange("p (v t) -> p v t", v=PW)
    for m in range(8):
        nc.sync.dma_start(
            xpad_v[:, 1 + m * 8:1 + (m + 1) * 8, 1:1 + T],
            xbf_d.ap()[:, m * 8:(m + 1) * 8, :])
    x_sb = persist.tile([IN, NPOS], F32)
    nc.sync.dma_start(x_sb[:].rearrange("p (v t) -> p v t", v=V), x_d.ap())

    opad = persist.tile([CH3, PW * PW], F32)
    nc.gpsimd.memset(opad[:], 0.0)
    opad_v = opad[:].rearrange("p (v t) -> p v t", v=PW)

    # ---- Phase 1: qkv conv (9-tap accumulating bf16 matmuls) ----
    for m in range(8):
        ps = psum1.tile([CH3, 512], F32, tag="ps1")
        for tap in range(9):
            dy, dx = tap // 3, tap % 3
            rhs = xpad_v[:, m * 8 + dy: m * 8 + dy + 8, dx: dx + T]
            nc.tensor.matmul(
                ps[:], wqkv[:, tap * CH3:(tap + 1) * CH3], rhs,
                start=(tap == 0), stop=(tap == 8))
        qkv_t = pio.tile([CH3, 512], F32, tag="qkv_t")
        nc.scalar.activation(qkv_t[:], ps[:], AF.Identity, bias=bqkv[:])
        nc.sync.dma_start(
            qkv_h.ap()[:, m * 8:(m + 1) * 8, :],
            qkv_t[:].rearrange("p (v t) -> p v t", v=8))

    # ---- Phase 2: cc branch in [t, (v,c)] layout ----
    # q_ccT[t, v*16+c] = qf[(t*64+v)*16+c]; same for k; value chunk is the
    # (v,t)-swapped one: v_ccT[t, v*16+j] = vf[(v*64+t)*16+j].
    q_ccT = persist.tile([T, V * C], F32)
    k_ccT = persist.tile([T, V * C], F32)
    v_ccT = persist.tile([T, V * C], F32)
    qflat = qkv_h.ap()[0:C].flatten()
    kflat = qkv_h.ap()[C:2 * C].flatten()
    vflat = qkv_h.ap()[2 * C:3 * C].flatten()
    nc.sync.dma_start(q_ccT[:], qflat.rearrange("(t u) -> t u", t=T))
    nc.sync.dma_start(k_ccT[:], kflat.rearrange("(t u) -> t u", t=T))
    nc.sync.dma_start(
        v_ccT[:].rearrange("t (v j) -> t v j", v=V),
        vflat.rearrange("(v t j) -> t v j", v=V, t=T))

    # S_ccT/W_ccT free layout (c*64 + v) so per-block slices are contiguous
    v_ccbf = persist.tile([T, V * C], BF16)
    nc.scalar.activation(v_ccbf[:], v_ccT[:], AF.Identity)

    S_ccT = persist.tile([T, C * V], F32)
    W_ccT = persist.tile([T, C * V], F32)
    q_v = q_ccT[:].rearrange("t (v c) -> t v c", v=V)
    k_v = k_ccT[:].rearrange("t (v c) -> t v c", v=V)
    v_v = v_ccbf[:].rearrange("t (v c) -> t v c", v=V)
    S_vw = S_ccT[:].rearrange("t (c v) -> t v c", c=C)
    W_vw = W_ccT[:].rearrange("t (c v) -> t v c", c=C)
    NV = 16  # v-chunk width
    for ch in range(V // NV):
        vs = slice(ch * NV, (ch + 1) * NV)
        qs = q_v[:, vs]
        ks = k_v[:, vs]
        vv = v_v[:, vs]
        Pc = pP.tile([128, 4096], F32, tag="P")
        Pv = Pc[0:T, :].rearrange("t (v c j) -> t v c j", v=NV, c=C)
        nc.vector.tensor_mul(
            Pv,
            qs.unsqueeze(3).broadcast_to([T, NV, C, C]),
            ks.unsqueeze(2).broadcast_to([T, NV, C, C]))
        Gc = pG.tile([128, 4096], BF16, tag="G")
        Gv = Gc[0:T, :].rearrange("t (v c j) -> t v c j", v=NV, c=C)
        nc.scalar.activation(Gv, Pv, AF.Exp)
        nc.vector.tensor_reduce(S_vw[:, vs], Gv, axis=AX.X, op=ALU.add)
        Hc = pP.tile([128, 4096], BF16, tag="H")
        Hv = Hc[0:T, :].rearrange("t (v c j) -> t v c j", v=NV, c=C)
        nc.gpsimd.tensor_mul(
            Hv, Gv, vv.unsqueeze(2).broadcast_to([T, NV, C, C]))
        nc.vector.tensor_reduce(W_vw[:, vs], Hv, axis=AX.X, op=ALU.add)

    # PE-transpose [64,(c2,v)] -> [(c2,v),64] per block; stash in SBUF
    Scc_all = persist.tile([128, 8 * T], F32)
    Wcc_all = persist.tile([128, 8 * T], F32)
    for b in range(8):
        fs = slice(b * 128, (b + 1) * 128)
        Sp = psumT.tile([128, T], F32, tag=f"S2{b % 2}", name=f"Sp{b}")
        nc.tensor.matmul(Sp[:], S_ccT[:, fs], ident[0:T, :],
                         is_transpose=True)
        nc.scalar.activation(Scc_all[:, b * T:(b + 1) * T], Sp[:],
                             AF.Identity)
        Wp = psumT.tile([128, T], F32, tag=f"W2{b % 2}", name=f"Wp{b}")
        nc.tensor.matmul(Wp[:], W_ccT[:, fs], ident[0:T, :],
                         is_transpose=True)
        nc.scalar.activation(Wcc_all[:, b * T:(b + 1) * T], Wp[:],
                             AF.Identity)

    # persistent per-row sums for branch A
    S_all = persist.tile([128, 8 * T], F32)
    W_all = persist.tile([128, 8 * T], F32)

    # ---- Phase 3+4: branch A block + combine, interleaved ----
    for b in range(8):
        Qb = pqkv.tile([128, T], F32, tag="Qb")
        nc.sync.dma_start(Qb[:], qkv_h.ap()[2 * b:2 * b + 2])
        Kb = pqkv.tile([128, T], F32, tag="Kb")
        nc.sync.dma_start(Kb[:], qkv_h.ap()[C + 2 * b:C + 2 * b + 2])
        Vb = pqkv.tile([128, T], F32, tag="Vb")
        nc.sync.dma_start(Vb[:], qkv_h.ap()[2 * C + 2 * b:2 * C + 2 * b + 2])
        Vbf = pqkv.tile([128, T], BF16, tag="Vbf")
        nc.scalar.activation(Vbf[:], Vb[:], AF.Identity)

        P = pP.tile([128, 4096], F32, tag="P")
        Pv3 = P[:].rearrange("p (x j) -> p x j", x=T)
        nc.vector.tensor_mul(
            Pv3,
            Qb[:].unsqueeze(2).broadcast_to([128, T, T]),
            Kb[:].unsqueeze(1).broadcast_to([128, T, T]))
        G = pG.tile([128, 4096], BF16, tag="G")
        Gv3 = G[:].rearrange("p (x j) -> p x j", x=T)
        nc.scalar.activation(G[:], P[:], AF.Exp)
        Sb = S_all[:, b * T:(b + 1) * T]
        Wb = W_all[:, b * T:(b + 1) * T]
        nc.vector.tensor_reduce(Sb, Gv3, axis=AX.X, op=ALU.add)
        H = pP.tile([128, 4096], BF16, tag="H")
        Hv3 = H[:].rearrange("p (x j) -> p x j", x=T)
        nc.gpsimd.tensor_mul(
            Hv3, Gv3, Vbf[:].unsqueeze(1).broadcast_to([128, T, T]))
        nc.vector.tensor_reduce(Wb, Hv3, axis=AX.X, op=ALU.add)

        # vv terms via on-chip PE transposes.  Transpose-mode matmuls must
        # write a fresh PSUM tile at partition 0 / offset 0, so each (c2)
        # half gets its own tile.
        S2h = [psumT.tile([T, T], F32, tag=f"S2{c2}", name=f"S2h{c2}_{b}")
               for c2 in range(2)]
        W2h = [psumT.tile([T, T], F32, tag=f"W2{c2}", name=f"W2h{c2}_{b}")
               for c2 in range(2)]
        for c2 in range(2):
            rows = slice(c2 * 64, (c2 + 1) * 64)
            nc.tensor.matmul(S2h[c2][:], Sb[rows], ident[rows, :],
                             is_transpose=True)
            nc.tensor.matmul(W2h[c2][:], Wb[rows], ident[rows, :],
                             is_transpose=True)

        Z = pcomb.tile([128, T], F32, tag="Z")
        R = pcomb.tile([128, T], F32, tag="R")
        Ov = pcomb.tile([128, T], F32, tag="Ov")
        for c2 in range(2):
            rows = slice(c2 * 64, (c2 + 1) * 64)
            nc.vector.tensor_add(Z[rows, :], Sb[rows], S2h[c2][:])
        nc.vector.tensor_add(Z[:], Z[:], Scc_all[:, b * T:(b + 1) * T])
        nc.vector.reciprocal(R[:], Z[:])
        for c2 in range(2):
            rows = slice(c2 * 64, (c2 + 1) * 64)
            nc.vector.tensor_mul(Ov[rows, :], W2h[c2][:], R[rows, :])

        Oc = pcomb.tile([128, T], F32, tag="Oc")
        nc.vector.tensor_mul(Oc[:], Wcc_all[:, b * T:(b + 1) * T], R[:])
        Ot = pcomb.tile([128, T], F32, tag="Ot")
        nc.vector.tensor_mul(Ot[:], Wb, R[:])

        for t_, off in ((Oc, 0), (Ov, C), (Ot, 2 * C)):
            nc.sync.dma_start(ocat_h.ap()[off + 2 * b:off + 2 * b + 2], t_[:])

    # ---- Phase 5: reverse conv + residual (bf16 matmuls) ----
    nc.sync.dma_start(opad_v[:, 1:1 + V, 1:1 + T], ocat_h.ap())
    opad_bf = persist.tile([CH3, PW * PW], BF16)
    nc.scalar.activation(opad_bf[:], opad[:], AF.Identity)
    opbf_v = opad_bf[:].rearrange("p (v t) -> p v t", v=PW)

    for m in range(8):
        ps2 = psum2.tile([IN, 512], F32, tag="ps2")
        for tap in range(9):
            dy, dx = tap // 3, tap % 3
            rhs = opbf_v[:, m * 8 + dy: m * 8 + dy + 8, dx: dx + T]
            nc.tensor.matmul(
                ps2[:], wr[:, tap * IN:(tap + 1) * IN], rhs,
                start=(tap == 0), stop=(tap == 8))
        o_sb = pio.tile([IN, 512], F32, tag="o_sb")
        nc.vector.scalar_tensor_tensor(
            o_sb[:], ps2[:], gb[:], x_sb[:, m * 512:(m + 1) * 512],
            op0=ALU.add, op1=ALU.add)
        nc.sync.dma_start(out_d.ap()[:, m * 8:(m + 1) * 8, :],
                          o_sb[:].rearrange("p (v t) -> p v t", v=8))

    ctx.close()


_NC_CACHE = {}


def _get_program(niter=1):
    if niter not in _NC_CACHE:
        _NC_CACHE[niter] = _build_program(niter)
    return _NC_CACHE[niter]


def _host_weights(wq, bq, wk, bk, wv, bv, wcr, bcr, wvr, bvr, wtr, btr, gamma):
    g = np.float32(np.asarray(gamma).reshape(-1)[0])
    wf = np.concatenate([wq, wk, wv], axis=0)  # [48, 128, 3, 3]
    wqkv = np.ascontiguousarray(
        wf.transpose(1, 2, 3, 0).reshape(IN, 9 * CH3)).astype(NPBF)
    bqkv = np.concatenate([bq, bk, bv]).reshape(CH3, 1).astype(np.float32)
    wrf = np.concatenate([wcr, wvr, wtr], axis=1) * g  # [128, 48, 3, 3]
    wr_ = np.ascontiguousarray(
        wrf.transpose(1, 2, 3, 0).reshape(CH3, 9 * IN)).astype(NPBF)
    gb = (g * (bcr + bvr + btr)).reshape(IN, 1).astype(np.float32)
    return wqkv, bqkv, wr_, gb


def kernel(x, wq, bq, wk, bk, wv, bv, wcr, bcr, wvr, bvr, wtr, btr, gamma,
           _trace=False, _niter=1):
    nc = _get_program(_niter)
    wqkv, bqkv, wr_, gb = _host_weights(
        wq, bq, wk, bk, wv, bv, wcr, bcr, wvr, bvr, wtr, btr, gamma)
    x = np.asarray(x, dtype=np.float32)
    ident = np.tile(np.eye(T, dtype=np.float32), (2, 1))
    in_maps = [
        {"x": np.ascontiguousarray(x[i]),
         "xbf": np.ascontiguousarray(x[i]).astype(NPBF),
         "wqkv": wqkv, "bqkv": bqkv,
         "wr": wr_, "gb": gb, "ident": ident}
        for i in range(8)
    ]
    res = run_bass_kernel_spmd(nc, in_maps, list(range(8)), trace=_trace)
    out = np.stack([res.results[i]["out"] for i in range(8)]).astype(np.float32)
    if _trace:
        kernel.last_exec_time_ns = res.exec_time_ns
        kernel.last_results = res
    return out


# revision 20
# speedup vs baseline: 2.2421x; 2.2421x over previous
"""Trainium2 Bass kernel for ChannelCrissCrossAttention.

Shapes (hardcoded): B=8, IN=128, C=16, V=T=64.
Sharding: pure data parallel, one batch element per NeuronCore (8 cores).

Math notes (derived from the reference's reshape/view semantics):
  q,k,v = conv3x3(x; wq/wk/wv)  -> [C, V, T] per batch element.
  - tt branch:  e_tt[c,v,t,j] = q[c,v,t] * k[c,v,j]
  - vv branch:  e_vv[c,v,t,j] = q[c,t,v] * k[c,t,j]  == e_tt[c,t,v,j]
    (the reference's .reshape on a [B,C,V,T] buffer makes vv the exact
     spatial transpose of tt, and its value rows coincide as well), so
    one pass over rows r=(c,y) gives both branches:
      G_r[x,j] = exp(qr[x]*kr[j]);  S_r[x] = sum_j G;  W_r[x] = sum_j G*vr[j]
  - cc branch: with rows r'=(t,v) over the flat [C*V*T] buffers
    (qf chunked by 16): logits qf[r'*16+c]*qf-like kf[r'*16+j], value
    chunk vf[(v*64+t)*16+j].  Computed here in a [t-partition, (v,c)]
    layout so the per-block (c,v,t) outputs fall out of a single PE
    transpose per block (no 4-byte-element DMA transposes).
  Z = S_tt + S_vv + S_cc (joint softmax denominator); the three outputs are
  W_*/Z, then one stacked reverse conv (48->128) + gamma*out + x.

Convs run in bf16 (PSUM accumulates in f32); everything else f32.
"""

import sys

sys.path.insert(0, "/opt/trn_rl_repo")

import ml_dtypes
import numpy as np

import concourse.bass as bass
import concourse.tile as tile
from concourse import bacc, mybir
from concourse.bass_utils import run_bass_kernel_spmd

F32 = mybir.dt.float32
BF16 = mybir.dt.bfloat16
AF = mybir.ActivationFunctionType
ALU = mybir.AluOpType
AX = mybir.AxisListType

IN, C, V, T = 128, 16, 64, 64
CH3 = 3 * C  # 48
NPOS = V * T  # 4096
PW = 66  # padded spatial width
NPBF = np.dtype(ml_dtypes.bfloat16)


def _build_program(niter=1):
    nc = bacc.Bacc("TRN2", target_bir_lowering=False, debug=False)

    x_d = nc.dram_tensor("x", [IN, V, T], F32, kind="ExternalInput")
    xbf_d = nc.dram_tensor("xbf", [IN, V, T], BF16, kind="ExternalInput")
    wqkv_d = nc.dram_tensor("wqkv", [IN, 9 * CH3], BF16, kind="ExternalInput")
    bqkv_d = nc.dram_tensor("bqkv", [CH3, 1], F32, kind="ExternalInput")
    wr_d = nc.dram_tensor("wr", [CH3, 9 * IN], BF16, kind="ExternalInput")
    gb_d = nc.dram_tensor("gb", [IN, 1], F32, kind="ExternalInput")
    id_d = nc.dram_tensor("ident", [2 * T, T], F32, kind="ExternalInput")
    out_d = nc.dram_tensor("out", [IN, V, T], F32, kind="ExternalOutput")

    # HBM bounce buffers (SBUF->SBUF DMAs cannot cross the partition/free
    # axis boundary, so layout changes bounce through DRAM)
    qkv_h = nc.dram_tensor("qkv_h", [CH3, V, T], F32)
    ocat_h = nc.dram_tensor("ocat_h", [CH3, V, T], F32)

    with tile.TileContext(nc) as tc:
        if niter == 1:
            _body(nc, tc, x_d, xbf_d, wqkv_d, bqkv_d, wr_d, gb_d, id_d, out_d,
                  qkv_h, ocat_h)
        else:
            # hardware loop: the N-iteration program has the same
            # instruction count as 1-iteration, so the (tN - t1) wall
            # delta isolates true marginal device-execution time instead
            # of NEFF load/dispatch overhead (which scales with the
            # instruction stream size).
            with tc.For_i(0, niter):
                _body(nc, tc, x_d, xbf_d, wqkv_d, bqkv_d, wr_d, gb_d, id_d,
                      out_d, qkv_h, ocat_h)

    nc.compile()
    return nc


def _body(nc, tc, x_d, xbf_d, wqkv_d, bqkv_d, wr_d, gb_d, id_d, out_d,
          qkv_h, ocat_h):
    from contextlib import ExitStack
    ctx = ExitStack()
    persist = ctx.enter_context(tc.tile_pool(name="persist", bufs=1))
    pio = ctx.enter_context(tc.tile_pool(name="pio", bufs=2))
    pqkv = ctx.enter_context(tc.tile_pool(name="pqkv", bufs=2))
    pP = ctx.enter_context(tc.tile_pool(name="pP", bufs=2))
    pG = ctx.enter_context(tc.tile_pool(name="pG", bufs=2))
    pF = ctx.enter_context(tc.tile_pool(name="pF", bufs=2))
    pcomb = ctx.enter_context(tc.tile_pool(name="pcomb", bufs=2))
    psum1 = ctx.enter_context(
        tc.tile_pool(name="psum1", bufs=2, space=bass.MemorySpace.PSUM))
    psum2 = ctx.enter_context(
        tc.tile_pool(name="psum2", bufs=2, space=bass.MemorySpace.PSUM))
    psumT = ctx.enter_context(
        tc.tile_pool(name="psumT", bufs=1, space=bass.MemorySpace.PSUM))

    # ---- Phase 0: weights + padded bf16 input + f32 residual copy ----
    wqkv = persist.tile([IN, 9 * CH3], BF16)
    nc.sync.dma_start(wqkv[:], wqkv_d.ap())
    bqkv = persist.tile([CH3, 1], F32)
    nc.sync.dma_start(bqkv[:], bqkv_d.ap())
    wr = persist.tile([CH3, 9 * IN], BF16)
    nc.sync.dma_start(wr[:], wr_d.ap())
    gb = persist.tile([IN, 1], F32)
    nc.sync.dma_start(gb[:], gb_d.ap())
    ident = persist.tile([2 * T, T], F32)
    nc.sync.dma_start(ident[:], id_d.ap())

    xpad = persist.tile([IN, PW * PW], BF16)
    nc.gpsimd.memset(xpad[:], 0.0)
    xpad_v = xpad[:].rearrange("p (v t) -> p v t", v=PW)
    for m in range(8):
        nc.sync.dma_start(
            xpad_v[:, 1 + m * 8:1 + (m + 1) * 8, 1:1 + T],
            xbf_d.ap()[:, m * 8:(m + 1) * 8, :])
    x_sb = persist.tile([IN, NPOS], F32)
    nc.sync.dma_start(x_sb[:].rearrange("p (v t) -> p v t", v=V), x_d.ap())

    opad = persist.tile([CH3, PW * PW], F32)
    nc.gpsimd.memset(opad[:], 0.0)
    opad_v = opad[:].rearrange("p (v t) -> p v t", v=PW)

    # ---- Phase 1: qkv conv (9-tap accumulating bf16 matmuls) ----
    for m in range(8):
        ps = psum1.tile([CH3, 512], F32, tag="ps1")
        for tap in range(9):
            dy, dx = tap // 3, tap % 3
            rhs = xpad_v[:, m * 8 + dy: m * 8 + dy + 8, dx: dx + T]
            nc.tensor.matmul(
                ps[:], wqkv[:, tap * CH3:(tap + 1) * CH3], rhs,
                start=(tap == 0), stop=(tap == 8))
        qkv_t = pio.tile([CH3, 512], F32, tag="qkv_t")
        nc.scalar.activation(qkv_t[:], ps[:], AF.Identity, bias=bqkv[:])
        nc.sync.dma_start(
            qkv_h.ap()[:, m * 8:(m + 1) * 8, :],
            qkv_t[:].rearrange("p (v t) -> p v t", v=8))

    # ---- Phase 2: cc branch in [t, (v,c)] layout ----
    # q_ccT[t, v*16+c] = qf[(t*64+v)*16+c]; same for k; value chunk is the
    # (v,t)-swapped one: v_ccT[t, v*16+j] = vf[(v*64+t)*16+j].
    q_ccT = persist.tile([T, V * C], F32)
    k_ccT = persist.tile([T, V * C], F32)
    v_ccT = persist.tile([T, V * C], F32)
    qflat = qkv_h.ap()[0:C].flatten()
    kflat = qkv_h.ap()[C:2 * C].flatten()
    vflat = qkv_h.ap()[2 * C:3 * C].flatten()
    nc.sync.dma_start(q_ccT[:], qflat.rearrange("(t u) -> t u", t=T))
    nc.sync.dma_start(k_ccT[:], kflat.rearrange("(t u) -> t u", t=T))
    nc.sync.dma_start(
        v_ccT[:].rearrange("t (v j) -> t v j", v=V),
        vflat.rearrange("(v t j) -> t v j", v=V, t=T))

    # S_ccT/W_ccT free layout (c*64 + v) so per-block slices are contiguous
    v_ccbf = persist.tile([T, V * C], BF16)
    nc.scalar.activation(v_ccbf[:], v_ccT[:], AF.Identity)

    S_ccT = persist.tile([T, C * V], F32)
    W_ccT = persist.tile([T, C * V], F32)
    q_v = q_ccT[:].rearrange("t (v c) -> t v c", v=V)
    k_v = k_ccT[:].rearrange("t (v c) -> t v c", v=V)
    v_v = v_ccbf[:].rearrange("t (v c) -> t v c", v=V)
    S_vw = S_ccT[:].rearrange("t (c v) -> t v c", c=C)
    W_vw = W_ccT[:].rearrange("t (c v) -> t v c", c=C)
    NV = 16  # v-chunk width
    for ch in range(V // NV):
        vs = slice(ch * NV, (ch + 1) * NV)
        qs = q_v[:, vs]
        ks = k_v[:, vs]
        vv = v_v[:, vs]
        Pc = pP.tile([128, 4096], F32, tag="P")
        Pv = Pc[0:T, :].rearrange("t (v c j) -> t v c j", v=NV, c=C)
        nc.vector.tensor_mul(
            Pv,
            qs.unsqueeze(3).broadcast_to([T, NV, C, C]),
            ks.unsqueeze(2).broadcast_to([T, NV, C, C]))
        Gc = pG.tile([128, 4096], BF16, tag="G")
        Gv = Gc[0:T, :].rearrange("t (v c j) -> t v c j", v=NV, c=C)
        nc.scalar.activation(Gv, Pv, AF.Exp)
        F1 = pF.tile([128, 2048], BF16, tag="F1")
        f1 = F1[0:T, :].rearrange("t (v c j) -> t v c j", v=NV, c=C)
        nc.vector.tensor_add(f1, Gv[:, :, :, 0:8], Gv[:, :, :, 8:16])
        F2 = pF.tile([128, 1024], BF16, tag="F2")
        f2 = F2[0:T, :].rearrange("t (v c j) -> t v c j", v=NV, c=C)
        nc.vector.tensor_add(f2, f1[:, :, :, 0:4], f1[:, :, :, 4:8])
        nc.vector.tensor_reduce(S_vw[:, vs], f2, axis=AX.X, op=ALU.add)
        Hc = pP.tile([128, 4096], BF16, tag="H")
        Hv = Hc[0:T, :].rearrange("t (v c j) -> t v c j", v=NV, c=C)
        nc.vector.tensor_mul(
            Hv, Gv, vv.unsqueeze(2).broadcast_to([T, NV, C, C]))
        F1w = pF.tile([128, 2048], BF16, tag="F1")
        f1w = F1w[0:T, :].rearrange("t (v c j) -> t v c j", v=NV, c=C)
        nc.vector.tensor_add(f1w, Hv[:, :, :, 0:8], Hv[:, :, :, 8:16])
        F2w = pF.tile([128, 1024], BF16, tag="F2")
        f2w = F2w[0:T, :].rearrange("t (v c j) -> t v c j", v=NV, c=C)
        nc.vector.tensor_add(f2w, f1w[:, :, :, 0:4], f1w[:, :, :, 4:8])
        nc.vector.tensor_reduce(W_vw[:, vs], f2w, axis=AX.X, op=ALU.add)

    # PE-transpose [64,(c2,v)] -> [(c2,v),64] per block; stash in SBUF
    Scc_all = persist.tile([128, 8 * T], F32)
    Wcc_all = persist.tile([128, 8 * T], F32)
    for b in range(8):
        fs = slice(b * 128, (b + 1) * 128)
        Sp = psumT.tile([128, T], F32, tag=f"S2{b % 2}", name=f"Sp{b}")
        nc.tensor.matmul(Sp[:], S_ccT[:, fs], ident[0:T, :],
                         is_transpose=True)
        nc.scalar.activation(Scc_all[:, b * T:(b + 1) * T], Sp[:],
                             AF.Identity)
        Wp = psumT.tile([128, T], F32, tag=f"W2{b % 2}", name=f"Wp{b}")
        nc.tensor.matmul(Wp[:], W_ccT[:, fs], ident[0:T, :],
                         is_transpose=True)
        nc.scalar.activation(Wcc_all[:, b * T:(b + 1) * T], Wp[:],
                             AF.Identity)

    # persistent per-row sums for branch A
    S_all = persist.tile([128, 8 * T], F32)
    W_all = persist.tile([128, 8 * T], F32)

    # ---- Phase 3+4: branch A block + combine, interleaved ----
    for b in range(8):
        Qb = pqkv.tile([128, T], F32, tag="Qb")
        nc.sync.dma_start(Qb[:], qkv_h.ap()[2 * b:2 * b + 2])
        Kb = pqkv.tile([128, T], F32, tag="Kb")
        nc.sync.dma_start(Kb[:], qkv_h.ap()[C + 2 * b:C + 2 * b + 2])
        Vb = pqkv.tile([128, T], F32, tag="Vb")
        nc.sync.dma_start(Vb[:], qkv_h.ap()[2 * C + 2 * b:2 * C + 2 * b + 2])
        Vbf = pqkv.tile([128, T], BF16, tag="Vbf")
        nc.scalar.activation(Vbf[:], Vb[:], AF.Identity)

        P = pP.tile([128, 4096], F32, tag="P")
        Pv3 = P[:].rearrange("p (x j) -> p x j", x=T)
        nc.vector.tensor_mul(
            Pv3,
            Qb[:].unsqueeze(2).broadcast_to([128, T, T]),
            Kb[:].unsqueeze(1).broadcast_to([128, T, T]))
        G = pG.tile([128, 4096], BF16, tag="G")
        Gv3 = G[:].rearrange("p (x j) -> p x j", x=T)
        nc.scalar.activation(G[:], P[:], AF.Exp)
        Sb = S_all[:, b * T:(b + 1) * T]
        Wb = W_all[:, b * T:(b + 1) * T]
        # two halving folds (bf16, 4 elem/cyc) before the 1 elem/cyc
        # TensorReduce: 4096-read reduce becomes 2048+1024 TT + 1024 TR.
        F1 = pF.tile([128, 2048], BF16, tag="F1")
        f1 = F1[:].rearrange("p (x j) -> p x j", x=T)
        nc.vector.tensor_add(f1, Gv3[:, :, 0:32], Gv3[:, :, 32:64])
        F2 = pF.tile([128, 1024], BF16, tag="F2")
        f2 = F2[:].rearrange("p (x j) -> p x j", x=T)
        nc.vector.tensor_add(f2, f1[:, :, 0:16], f1[:, :, 16:32])
        nc.vector.tensor_reduce(Sb, f2, axis=AX.X, op=ALU.add)
        H = pP.tile([128, 4096], BF16, tag="H")
        Hv3 = H[:].rearrange("p (x j) -> p x j", x=T)
        nc.vector.tensor_mul(
            Hv3, Gv3, Vbf[:].unsqueeze(1).broadcast_to([128, T, T]))
        F1w = pF.tile([128, 2048], BF16, tag="F1")
        f1w = F1w[:].rearrange("p (x j) -> p x j", x=T)
        nc.vector.tensor_add(f1w, Hv3[:, :, 0:32], Hv3[:, :, 32:64])
        F2w = pF.tile([128, 1024], BF16, tag="F2")
        f2w = F2w[:].rearrange("p (x j) -> p x j", x=T)
        nc.vector.tensor_add(f2w, f1w[:, :, 0:16], f1w[:, :, 16:32])
        nc.vector.tensor_reduce(Wb, f2w, axis=AX.X, op=ALU.add)

        # vv terms via on-chip PE transposes.  Transpose-mode matmuls must
        # write a fresh PSUM tile at partition 0 / offset 0, so each (c2)
        # half gets its own tile.
        S2h = [psumT.tile([T, T], F32, tag=f"S2{c2}", name=f"S2h{c2}_{b}")
               for c2 in range(2)]
        W2h = [psumT.tile([T, T], F32, tag=f"W2{c2}", name=f"W2h{c2}_{b}")
               for c2 in range(2)]
        for c2 in range(2):
            rows = slice(c2 * 64, (c2 + 1) * 64)
            nc.tensor.matmul(S2h[c2][:], Sb[rows], ident[rows, :],
                             is_transpose=True)
            nc.tensor.matmul(W2h[c2][:], Wb[rows], ident[rows, :],
                             is_transpose=True)

        Z = pcomb.tile([128, T], F32, tag="Z")
        R = pcomb.tile([128, T], F32, tag="R")
        Ov = pcomb.tile([128, T], F32, tag="Ov")
        for c2 in range(2):
            rows = slice(c2 * 64, (c2 + 1) * 64)
            nc.vector.tensor_add(Z[rows, :], Sb[rows], S2h[c2][:])
        nc.vector.tensor_add(Z[:], Z[:], Scc_all[:, b * T:(b + 1) * T])
        nc.vector.reciprocal(R[:], Z[:])
        for c2 in range(2):
            rows = slice(c2 * 64, (c2 + 1) * 64)
            nc.vector.tensor_mul(Ov[rows, :], W2h[c2][:], R[rows, :])

        Oc = pcomb.tile([128, T], F32, tag="Oc")
        nc.vector.tensor_mul(Oc[:], Wcc_all[:, b * T:(b + 1) * T], R[:])
        Ot = pcomb.tile([128, T], F32, tag="Ot")
        nc.vector.tensor_mul(Ot[:], Wb, R[:])

        for t_, off in ((Oc, 0), (Ov, C), (Ot, 2 * C)):
            nc.sync.dma_start(ocat_h.ap()[off + 2 * b:off + 2 * b + 2], t_[:])

    # ---- Phase 5: reverse conv + residual (bf16 matmuls) ----
    nc.sync.dma_start(opad_v[:, 1:1 + V, 1:1 + T], ocat_h.ap())
    opad_bf = persist.tile([CH3, PW * PW], BF16)
    nc.scalar.activation(opad_bf[:], opad[:], AF.Identity)
    opbf_v = opad_bf[:].rearrange("p (v t) -> p v t", v=PW)

    for m in range(8):
        ps2 = psum2.tile([IN, 512], F32, tag="ps2")
        for tap in range(9):
            dy, dx = tap // 3, tap % 3
            rhs = opbf_v[:, m * 8 + dy: m * 8 + dy + 8, dx: dx + T]
            nc.tensor.matmul(
                ps2[:], wr[:, tap * IN:(tap + 1) * IN], rhs,
                start=(tap == 0), stop=(tap == 8))
        o_sb = pio.tile([IN, 512], F32, tag="o_sb")
        nc.vector.scalar_tensor_tensor(
            o_sb[:], ps2[:], gb[:], x_sb[:, m * 512:(m + 1) * 512],
            op0=ALU.add, op1=ALU.add)
        nc.sync.dma_start(out_d.ap()[:, m * 8:(m + 1) * 8, :],
                          o_sb[:].rearrange("p (v t) -> p v t", v=8))

    ctx.close()


_NC_CACHE = {}


def _get_program(niter=1):
    if niter not in _NC_CACHE:
        _NC_CACHE[niter] = _build_program(niter)
    return _NC_CACHE[niter]


def _host_weights(wq, bq, wk, bk, wv, bv, wcr, bcr, wvr, bvr, wtr, btr, gamma):
    g = np.float32(np.asarray(gamma).reshape(-1)[0])
    wf = np.concatenate([wq, wk, wv], axis=0)  # [48, 128, 3, 3]
    wqkv = np.ascontiguousarray(
        wf.transpose(1, 2, 3, 0).reshape(IN, 9 * CH3)).astype(NPBF)
    bqkv = np.concatenate([bq, bk, bv]).reshape(CH3, 1).astype(np.float32)
    wrf = np.concatenate([wcr, wvr, wtr], axis=1) * g  # [128, 48, 3, 3]
    wr_ = np.ascontiguousarray(
        wrf.transpose(1, 2, 3, 0).reshape(CH3, 9 * IN)).astype(NPBF)
    gb = (g * (bcr + bvr + btr)).reshape(IN, 1).astype(np.float32)
    return wqkv, bqkv, wr_, gb


def kernel(x, wq, bq, wk, bk, wv, bv, wcr, bcr, wvr, bvr, wtr, btr, gamma,
           _trace=False, _niter=1):
    nc = _get_program(_niter)
    wqkv, bqkv, wr_, gb = _host_weights(
        wq, bq, wk, bk, wv, bv, wcr, bcr, wvr, bvr, wtr, btr, gamma)
    x = np.asarray(x, dtype=np.float32)
    ident = np.tile(np.eye(T, dtype=np.float32), (2, 1))
    in_maps = [
        {"x": np.ascontiguousarray(x[i]),
         "xbf": np.ascontiguousarray(x[i]).astype(NPBF),
         "wqkv": wqkv, "bqkv": bqkv,
         "wr": wr_, "gb": gb, "ident": ident}
        for i in range(8)
    ]
    res = run_bass_kernel_spmd(nc, in_maps, list(range(8)), trace=_trace)
    out = np.stack([res.results[i]["out"] for i in range(8)]).astype(np.float32)
    if _trace:
        kernel.last_exec_time_ns = res.exec_time_ns
        kernel.last_results = res
    return out
